# revision 27
# baseline (speedup 1.0000x reference)
"""RWKV-style WKV attention kernel for 8 TRN2 NeuronCores.

Strategy (batch x channel parallel, zero collectives):
  - The 8 cores each handle 64 channels of ONE batch (2 batches x 4
    channel-groups). Per core: k/v/r via one PSUM-accumulated bf16 matmul
    chain (time-mix folded into pre-scaled stacked weights; the time-shift xs
    is a shifted window into the same x^T buffer), the WKV recurrence as ONE
    128-partition DVE tensor_tensor_scan (rows 0:64 = num, rows 64:128 =
    den), sigmoid folded into the denominator via e^{-r}, and a partial
    output projection (wkvsr @ Wo[:, shard].T) in bf16. The host sums the 4
    partial outputs per batch.
"""

import ml_dtypes
import numpy as np

import concourse.bass as bass
import concourse.mybir as mybir
import concourse.tile as tile
from concourse import bacc
from concourse.bass_utils import run_bass_kernel_spmd

B, T, D = 2, 512, 256
NCORES = 8
GROUPS = NCORES // B   # 4 channel groups
CLOC = D // GROUPS     # 64 channels per core
PP = 2 * CLOC          # 128 scan rows: num block then den block
F32 = mybir.dt.float32
BF16 = mybir.dt.bfloat16


def build_nc():
    nc = bacc.Bacc(None, target_bir_lowering=False)

    H = T + 1
    xT = nc.declare_dram_parameter("xT", [D, H], BF16, isOutput=False)
    wkvr = nc.declare_dram_parameter("wkvr", [2 * D, 3 * CLOC], BF16, isOutput=False)
    woT = nc.declare_dram_parameter("woT", [CLOC, D], BF16, isOutput=False)
    ab = nc.declare_dram_parameter("abcast", [PP, T], F32, isOutput=False)
    etf = nc.declare_dram_parameter("etf", [PP, 1], F32, isOutput=False)
    lnd = nc.declare_dram_parameter("lastnd", [PP, 1], F32, isOutput=False)
    outp = nc.declare_dram_parameter("out_part", [T, D], BF16, isOutput=True)
    ndl = nc.declare_dram_parameter("nd_last", [PP, 1], F32, isOutput=True)

    Exp = mybir.ActivationFunctionType.Exp
    mult = mybir.AluOpType.mult
    add = mybir.AluOpType.add

    with tile.TileContext(nc) as tc:
        with (
            tc.tile_pool(name="sb", bufs=1) as sb,
            tc.tile_pool(name="psk", bufs=2, space="PSUM") as psk,
            tc.tile_pool(name="pso", bufs=2, space="PSUM") as pso,
        ):
            # ---- input DMAs across the three DMA queues ----
            # weight lhsT, split so the k/v group (matmul 1) lands first
            wr = wkvr.rearrange("(k p) m -> p k m", k=4)
            wtA = sb.tile([128, 4, 2 * CLOC], BF16)
            for kt in (0, 2, 1, 3):  # match matmul consumption order
                nc.scalar.dma_start(out=wtA[:, kt, :],
                                    in_=wr[:, kt, 0 : 2 * CLOC])
            xt0 = sb.tile([128, H], BF16)
            xt1 = sb.tile([128, H], BF16)
            nc.sync.dma_start(out=xt0[0:64, :], in_=xT[0:64, :])
            nc.gpsimd.dma_start(out=xt0[64:128, :], in_=xT[64:128, :])
            nc.sync.dma_start(out=xt1[0:64, :], in_=xT[128:192, :])
            nc.gpsimd.dma_start(out=xt1[64:128, :], in_=xT[192:256, :])
            xt = [xt0, xt1]
            wtB = sb.tile([128, 4, CLOC], BF16)
            for kt in (0, 2, 1, 3):
                nc.scalar.dma_start(out=wtB[:, kt, :],
                                    in_=wr[:, kt, 2 * CLOC : 3 * CLOC])
            ab_t = sb.tile([PP, T], F32)
            nc.sync.dma_start(out=ab_t, in_=ab[:, :])
            wo_t = sb.tile([CLOC, D], BF16)
            nc.scalar.dma_start(out=wo_t, in_=woT[:, :])
            etf_t = sb.tile([PP, 1], F32)
            nc.scalar.dma_start(out=etf_t, in_=etf[:, :])

            # working tiles:
            #   u: rows 0:64 = e^k * v (num input), rows 64:128 = e^k (den)
            #   ndbuf col 0 = incoming state, cols 1..T = scanned num/den
            u = sb.tile([PP, T], F32)
            ndbuf = sb.tile([PP, T + 1], F32)
            er64 = sb.tile([CLOC, T], F32)   # e^{-r}
            nmrT = sb.tile([CLOC, T], F32)
            dnmT = sb.tile([CLOC, T], F32)
            dt = sb.tile([CLOC, T], F32)     # dnm * (1 + e^{-r})
            rcp = sb.tile([CLOC, T], F32)
            wsrb = sb.tile([CLOC, T], BF16)

            nc.gpsimd.dma_start(out=ndbuf[:, 0:1], in_=lnd[:, :])

            # warm the Exp table while DMAs run
            dummy = sb.tile([1, 1], F32)
            nc.vector.memset(dummy, 0.0)
            nc.scalar.activation(dummy, dummy, Exp)

            # ---- k/v and r projection matmuls (bf16 in, f32 accumulate) ----
            psA = psk.tile([2 * CLOC, T], F32, tag="kv")  # rows 0:64 k, 64:128 v
            psB = psk.tile([CLOC, T], F32, tag="r")
            # xt0-consuming matmuls first: xt1 halves land on the queues'
            # second slots, so kt order 0,2 (xt0) then 1,3 (xt1)
            KT_ORDER = (0, 2, 1, 3)
            for i, kt in enumerate(KT_ORDER):
                shift = 1 if kt < 2 else 0  # x view vs time-shifted xs view
                rhs = xt[kt % 2][:, shift : shift + T]
                nc.tensor.matmul(psA, wtA[:, kt, :], rhs,
                                 start=(i == 0), stop=(i == 3))
            for i, kt in enumerate(KT_ORDER):
                shift = 1 if kt < 2 else 0
                rhs = xt[kt % 2][:, shift : shift + T]
                nc.tensor.matmul(psB, wtB[:, kt, :], rhs,
                                 start=(i == 0), stop=(i == 3))

            # ---- activations: all on the Exp table ----
            nc.scalar.activation(u[CLOC:PP, :], psA[0:CLOC, :], Exp)  # e^k
            nc.scalar.activation(er64, psB[:, :], Exp, scale=-1.0)    # e^-r

            # u_num = e^k * v  (single 64-row op, SBUF read at base 64 with
            # a PSUM partner; falls back to 2x32 rows if HW disagrees)
            nc.vector.tensor_mul(u[0:CLOC, :], u[CLOC:PP, :], psA[CLOC:PP, :])

            # ---- one 128-row scan does num AND den ----
            nc.vector.tensor_tensor_scan(
                ndbuf[:, 1 : T + 1], ab_t, u, ndbuf[:, 0:1], mult, add)
            # dnm = ek*e^tf + den_prev (reads base 64, writes base 0)
            nc.vector.scalar_tensor_tensor(
                dnmT, u[CLOC:PP, :], etf_t[CLOC:PP, 0:1],
                ndbuf[CLOC:PP, 0:T], mult, add)
            # dt = dnm * (1 + e^-r): folds the sigmoid into the denominator
            nc.vector.scalar_tensor_tensor(dt, er64, 1.0, dnmT, add, mult)
            nc.vector.reciprocal_approx_fast(rcp, dt)
            nc.vector.scalar_tensor_tensor(
                nmrT, u[0:CLOC, :], etf_t[0:CLOC, 0:1],
                ndbuf[0:CLOC, 0:T], mult, add)
            for tp in range(2):
                cs = slice(tp * 256, (tp + 1) * 256)
                nc.vector.tensor_mul(wsrb[:, cs], nmrT[:, cs], rcp[:, cs])

            # ---- partial output projection (bf16, two t-blocks per bank) ----
            out_dma = [nc.sync, nc.scalar]
            for tp in range(2):
                po = pso.tile([128, 2, D], F32, tag="po")
                for h in range(2):
                    tt = tp * 2 + h
                    nc.tensor.matmul(
                        po[:, h, :], wsrb[:, tt * 128 : (tt + 1) * 128],
                        wo_t, start=True, stop=True,
                    )
                ob = sb.tile([128, 2, D], BF16, tag=f"ob{tp}")
                nc.any.tensor_copy(ob, po)
                for h in range(2):
                    tt = tp * 2 + h
                    out_dma[h].dma_start(
                        out=outp[tt * 128 : (tt + 1) * 128, :],
                        in_=ob[:, h, :],
                    )

            nc.scalar.dma_start(out=ndl[:, 0:1], in_=ndbuf[:, T : T + 1])

    nc.compile()
    return nc


_NC_CACHE = None


def _get_nc():
    global _NC_CACHE
    if _NC_CACHE is None:
        _NC_CACHE = build_nc()
    return _NC_CACHE


def prepare_in_maps(inputs):
    x = np.asarray(inputs["x"], np.float32)
    last_x = np.asarray(inputs["last_x"], np.float32)
    last_num = np.asarray(inputs["last_num"], np.float32)
    last_den = np.asarray(inputs["last_den"], np.float32)
    td = np.asarray(inputs["time_decay"], np.float32)
    tf = np.asarray(inputs["time_first"], np.float32)
    mk = np.asarray(inputs["time_mix_k"], np.float32).reshape(D)
    mv = np.asarray(inputs["time_mix_v"], np.float32).reshape(D)
    mr = np.asarray(inputs["time_mix_r"], np.float32).reshape(D)
    Wk = np.asarray(inputs["Wk"], np.float32)
    Wv = np.asarray(inputs["Wv"], np.float32)
    Wr = np.asarray(inputs["Wr"], np.float32)
    Wo = np.asarray(inputs["Wo"], np.float32)

    # per-batch x^T with the time-shift boundary column
    xTs = []
    for b in range(B):
        xT = np.empty((D, T + 1), np.float32)
        xT[:, 0] = last_x[b, 0, :]
        xT[:, 1:] = x[b].T
        xTs.append(xT.astype(ml_dtypes.bfloat16))

    a = np.exp(-np.exp(td, dtype=np.float64)).astype(np.float32)
    etf_full = np.exp(tf).astype(np.float32)

    in_maps = []
    for c in range(NCORES):
        b = c // GROUPS
        g = c % GROUPS
        sh = slice(g * CLOC, (g + 1) * CLOC)
        wx = np.concatenate(
            [(Wk[sh, :] * mk[None, :]).T,
             (Wv[sh, :] * mv[None, :]).T,
             (Wr[sh, :] * mr[None, :]).T], axis=1)
        wxs = np.concatenate(
            [(Wk[sh, :] * (1.0 - mk)[None, :]).T,
             (Wv[sh, :] * (1.0 - mv)[None, :]).T,
             (Wr[sh, :] * (1.0 - mr)[None, :]).T], axis=1)
        a_pp = np.tile(a[sh], 2)
        etf_pp = np.tile(etf_full[sh], 2)
        lnd_pp = np.concatenate([last_num[b, 0, sh], last_den[b, 0, sh]])
        in_maps.append({
            "xT": xTs[b],
            "wkvr": np.ascontiguousarray(
                np.concatenate([wx, wxs], axis=0)).astype(ml_dtypes.bfloat16),
            "woT": np.ascontiguousarray(Wo[:, sh].T).astype(ml_dtypes.bfloat16),
            "abcast": np.ascontiguousarray(
                np.repeat(a_pp[:, None], T, axis=1), dtype=np.float32),
            "etf": np.ascontiguousarray(etf_pp[:, None], dtype=np.float32),
            "lastnd": np.ascontiguousarray(lnd_pp[:, None], dtype=np.float32),
        })
    return in_maps


def postprocess(results, inputs):
    x = np.asarray(inputs["x"], np.float32)
    out = np.zeros((B, T, D), np.float32)
    num_l = np.empty((B, 1, D), np.float32)
    den_l = np.empty((B, 1, D), np.float32)
    for c in range(NCORES):
        b = c // GROUPS
        g = c % GROUPS
        sh = slice(g * CLOC, (g + 1) * CLOC)
        out[b] += results[c]["out_part"].reshape(T, D).astype(np.float32)
        nd = np.asarray(results[c]["nd_last"], np.float32).reshape(PP)
        num_l[b, 0, sh] = nd[0:CLOC]
        den_l[b, 0, sh] = nd[CLOC:PP]
    x_last = np.ascontiguousarray(x[:, -1:, :])
    return out, x_last, num_l, den_l


def kernel(**inputs):
    nc = _get_nc()
    in_maps = prepare_in_maps(inputs)
    res = run_bass_kernel_spmd(nc, in_maps, list(range(NCORES)))
    return postprocess(res.results, inputs)


# revision 28
# speedup vs baseline: 1.1413x; 1.1413x over previous
"""RWKV-style WKV attention kernel for 8 TRN2 NeuronCores.

Strategy (batch x channel parallel, zero collectives):
  - The 8 cores each handle 64 channels of ONE batch (2 batches x 4
    channel-groups). Per core: k/v/r via one PSUM-accumulated bf16 matmul
    chain (time-mix folded into pre-scaled stacked weights; the time-shift xs
    is a shifted window into the same x^T buffer), the WKV recurrence as ONE
    128-partition DVE tensor_tensor_scan (rows 0:64 = num, rows 64:128 =
    den), sigmoid folded into the denominator via e^{-r}, and a partial
    output projection (wkvsr @ Wo[:, shard].T) in bf16. The host sums the 4
    partial outputs per batch.
"""

import ml_dtypes
import numpy as np

import concourse.bass as bass
import concourse.mybir as mybir
import concourse.tile as tile
from concourse import bacc
from concourse.bass_utils import run_bass_kernel_spmd

B, T, D = 2, 512, 256
NCORES = 8
GROUPS = NCORES // B   # 4 channel groups
CLOC = D // GROUPS     # 64 channels per core
PP = 2 * CLOC          # 128 scan rows: num block then den block
F32 = mybir.dt.float32
BF16 = mybir.dt.bfloat16


def build_nc():
    nc = bacc.Bacc(None, target_bir_lowering=False)

    H = T + 1
    xT = nc.declare_dram_parameter("xT", [D, H], BF16, isOutput=False)
    wkvr = nc.declare_dram_parameter("wkvr", [2 * D, 3 * CLOC], BF16, isOutput=False)
    woT = nc.declare_dram_parameter("woT", [CLOC, D], BF16, isOutput=False)
    ab = nc.declare_dram_parameter("abcast", [PP, T], F32, isOutput=False)
    etf = nc.declare_dram_parameter("etf", [PP, 1], F32, isOutput=False)
    lnd = nc.declare_dram_parameter("lastnd", [PP, 1], F32, isOutput=False)
    outp = nc.declare_dram_parameter("out_part", [T, D], BF16, isOutput=True)
    ndl = nc.declare_dram_parameter("nd_last", [PP, 1], F32, isOutput=True)

    Exp = mybir.ActivationFunctionType.Exp
    mult = mybir.AluOpType.mult
    add = mybir.AluOpType.add

    with tile.TileContext(nc) as tc:
        with (
            tc.tile_pool(name="sb", bufs=1) as sb,
            tc.tile_pool(name="psk", bufs=2, space="PSUM") as psk,
            tc.tile_pool(name="pso", bufs=2, space="PSUM") as pso,
        ):
            # ---- input DMAs across the three DMA queues ----
            # weight lhsT, split so the k/v group (matmul 1) lands first
            wr = wkvr.rearrange("(k p) m -> p k m", k=4)
            wtA = sb.tile([128, 4, 2 * CLOC], BF16)
            for kt in (0, 2, 1, 3):  # match matmul consumption order
                nc.scalar.dma_start(out=wtA[:, kt, :],
                                    in_=wr[:, kt, 0 : 2 * CLOC])
            xt0 = sb.tile([128, H], BF16)
            xt1 = sb.tile([128, H], BF16)
            nc.sync.dma_start(out=xt0[0:64, :], in_=xT[0:64, :])
            nc.gpsimd.dma_start(out=xt0[64:128, :], in_=xT[64:128, :])
            nc.sync.dma_start(out=xt1[0:64, :], in_=xT[128:192, :])
            nc.gpsimd.dma_start(out=xt1[64:128, :], in_=xT[192:256, :])
            xt = [xt0, xt1]
            wtB = sb.tile([128, 4, CLOC], BF16)
            nc.scalar.dma_start(out=wtB, in_=wr[:, :, 2 * CLOC : 3 * CLOC])
            ab_t = sb.tile([PP, T], F32)
            nc.sync.dma_start(out=ab_t, in_=ab[:, :])
            wo_t = sb.tile([CLOC, D], BF16)
            nc.scalar.dma_start(out=wo_t, in_=woT[:, :])
            etf_t = sb.tile([PP, 1], F32)
            nc.scalar.dma_start(out=etf_t, in_=etf[:, :])

            # working tiles:
            #   u: rows 0:64 = e^k * v (num input), rows 64:128 = e^k (den)
            #   ndbuf col 0 = incoming state, cols 1..T = scanned num/den
            u = sb.tile([PP, T], F32)
            ndbuf = sb.tile([PP, T + 1], F32)
            er64 = sb.tile([CLOC, T], F32)   # e^{-r}
            nmrT = sb.tile([CLOC, T], F32)
            dnmT = sb.tile([CLOC, T], F32)
            dt = sb.tile([CLOC, T], F32)     # dnm * (1 + e^{-r})
            rcp = sb.tile([CLOC, T], F32)
            wsrb = sb.tile([CLOC, T], BF16)

            nc.gpsimd.dma_start(out=ndbuf[:, 0:1], in_=lnd[:, :])

            # warm the Exp table while DMAs run
            dummy = sb.tile([1, 1], F32)
            nc.vector.memset(dummy, 0.0)
            nc.scalar.activation(dummy, dummy, Exp)

            # ---- k/v and r projection matmuls (bf16 in, f32 accumulate) ----
            psA = psk.tile([2 * CLOC, T], F32, tag="kv")  # rows 0:64 k, 64:128 v
            psB = psk.tile([CLOC, T], F32, tag="r")
            # xt0-consuming matmuls first: xt1 halves land on the queues'
            # second slots, so kt order 0,2 (xt0) then 1,3 (xt1)
            KT_ORDER = (0, 2, 1, 3)
            for i, kt in enumerate(KT_ORDER):
                shift = 1 if kt < 2 else 0  # x view vs time-shifted xs view
                rhs = xt[kt % 2][:, shift : shift + T]
                nc.tensor.matmul(psA, wtA[:, kt, :], rhs,
                                 start=(i == 0), stop=(i == 3))
            for i, kt in enumerate(KT_ORDER):
                shift = 1 if kt < 2 else 0
                rhs = xt[kt % 2][:, shift : shift + T]
                nc.tensor.matmul(psB, wtB[:, kt, :], rhs,
                                 start=(i == 0), stop=(i == 3))

            # ---- activations: all on the Exp table ----
            nc.scalar.activation(u[CLOC:PP, :], psA[0:CLOC, :], Exp)  # e^k
            nc.scalar.activation(er64, psB[:, :], Exp, scale=-1.0)    # e^-r

            # u_num = e^k * v  (single 64-row op, SBUF read at base 64 with
            # a PSUM partner; falls back to 2x32 rows if HW disagrees)
            nc.vector.tensor_mul(u[0:CLOC, :], u[CLOC:PP, :], psA[CLOC:PP, :])

            # ---- one 128-row scan does num AND den ----
            nc.vector.tensor_tensor_scan(
                ndbuf[:, 1 : T + 1], ab_t, u, ndbuf[:, 0:1], mult, add)
            # dnm = ek*e^tf + den_prev (reads base 64, writes base 0)
            nc.vector.scalar_tensor_tensor(
                dnmT, u[CLOC:PP, :], etf_t[CLOC:PP, 0:1],
                ndbuf[CLOC:PP, 0:T], mult, add)
            # dt = dnm * (1 + e^-r): folds the sigmoid into the denominator
            nc.vector.scalar_tensor_tensor(dt, er64, 1.0, dnmT, add, mult)
            nc.vector.reciprocal_approx_fast(rcp, dt)
            nc.vector.scalar_tensor_tensor(
                nmrT, u[0:CLOC, :], etf_t[0:CLOC, 0:1],
                ndbuf[0:CLOC, 0:T], mult, add)
            nc.vector.tensor_mul(wsrb, nmrT, rcp)

            # ---- partial output projection (bf16, two t-blocks per bank) ----
            out_dma = [nc.sync, nc.scalar]
            for tp in range(2):
                po = pso.tile([128, 2, D], F32, tag="po")
                for h in range(2):
                    tt = tp * 2 + h
                    nc.tensor.matmul(
                        po[:, h, :], wsrb[:, tt * 128 : (tt + 1) * 128],
                        wo_t, start=True, stop=True,
                    )
                ob = sb.tile([128, 2, D], BF16, tag=f"ob{tp}")
                nc.any.tensor_copy(ob, po)
                for h in range(2):
                    tt = tp * 2 + h
                    out_dma[h].dma_start(
                        out=outp[tt * 128 : (tt + 1) * 128, :],
                        in_=ob[:, h, :],
                    )

            nc.scalar.dma_start(out=ndl[:, 0:1], in_=ndbuf[:, T : T + 1])

    nc.compile()
    return nc


_NC_CACHE = None


def _get_nc():
    global _NC_CACHE
    if _NC_CACHE is None:
        _NC_CACHE = build_nc()
    return _NC_CACHE


def prepare_in_maps(inputs):
    x = np.asarray(inputs["x"], np.float32)
    last_x = np.asarray(inputs["last_x"], np.float32)
    last_num = np.asarray(inputs["last_num"], np.float32)
    last_den = np.asarray(inputs["last_den"], np.float32)
    td = np.asarray(inputs["time_decay"], np.float32)
    tf = np.asarray(inputs["time_first"], np.float32)
    mk = np.asarray(inputs["time_mix_k"], np.float32).reshape(D)
    mv = np.asarray(inputs["time_mix_v"], np.float32).reshape(D)
    mr = np.asarray(inputs["time_mix_r"], np.float32).reshape(D)
    Wk = np.asarray(inputs["Wk"], np.float32)
    Wv = np.asarray(inputs["Wv"], np.float32)
    Wr = np.asarray(inputs["Wr"], np.float32)
    Wo = np.asarray(inputs["Wo"], np.float32)

    # per-batch x^T with the time-shift boundary column
    xTs = []
    for b in range(B):
        xT = np.empty((D, T + 1), np.float32)
        xT[:, 0] = last_x[b, 0, :]
        xT[:, 1:] = x[b].T
        xTs.append(xT.astype(ml_dtypes.bfloat16))

    a = np.exp(-np.exp(td, dtype=np.float64)).astype(np.float32)
    etf_full = np.exp(tf).astype(np.float32)

    in_maps = []
    for c in range(NCORES):
        b = c // GROUPS
        g = c % GROUPS
        sh = slice(g * CLOC, (g + 1) * CLOC)
        wx = np.concatenate(
            [(Wk[sh, :] * mk[None, :]).T,
             (Wv[sh, :] * mv[None, :]).T,
             (Wr[sh, :] * mr[None, :]).T], axis=1)
        wxs = np.concatenate(
            [(Wk[sh, :] * (1.0 - mk)[None, :]).T,
             (Wv[sh, :] * (1.0 - mv)[None, :]).T,
             (Wr[sh, :] * (1.0 - mr)[None, :]).T], axis=1)
        a_pp = np.tile(a[sh], 2)
        etf_pp = np.tile(etf_full[sh], 2)
        lnd_pp = np.concatenate([last_num[b, 0, sh], last_den[b, 0, sh]])
        in_maps.append({
            "xT": xTs[b],
            "wkvr": np.ascontiguousarray(
                np.concatenate([wx, wxs], axis=0)).astype(ml_dtypes.bfloat16),
            "woT": np.ascontiguousarray(Wo[:, sh].T).astype(ml_dtypes.bfloat16),
            "abcast": np.ascontiguousarray(
                np.repeat(a_pp[:, None], T, axis=1), dtype=np.float32),
            "etf": np.ascontiguousarray(etf_pp[:, None], dtype=np.float32),
            "lastnd": np.ascontiguousarray(lnd_pp[:, None], dtype=np.float32),
        })
    return in_maps


def postprocess(results, inputs):
    x = np.asarray(inputs["x"], np.float32)
    out = np.zeros((B, T, D), np.float32)
    num_l = np.empty((B, 1, D), np.float32)
    den_l = np.empty((B, 1, D), np.float32)
    for c in range(NCORES):
        b = c // GROUPS
        g = c % GROUPS
        sh = slice(g * CLOC, (g + 1) * CLOC)
        out[b] += results[c]["out_part"].reshape(T, D).astype(np.float32)
        nd = np.asarray(results[c]["nd_last"], np.float32).reshape(PP)
        num_l[b, 0, sh] = nd[0:CLOC]
        den_l[b, 0, sh] = nd[CLOC:PP]
    x_last = np.ascontiguousarray(x[:, -1:, :])
    return out, x_last, num_l, den_l


def kernel(**inputs):
    nc = _get_nc()
    in_maps = prepare_in_maps(inputs)
    res = run_bass_kernel_spmd(nc, in_maps, list(range(NCORES)))
    return postprocess(res.results, inputs)


# revision 29
# speedup vs baseline: 1.2023x; 1.0534x over previous
"""RWKV-style WKV attention kernel for 8 TRN2 NeuronCores.

Strategy (batch x channel parallel, zero collectives):
  - The 8 cores each handle 64 channels of ONE batch (2 batches x 4
    channel-groups). Per core: k/v/r via one PSUM-accumulated bf16 matmul
    chain (time-mix folded into pre-scaled stacked weights; the time-shift xs
    is a shifted window into the same x^T buffer), the WKV recurrence as ONE
    128-partition DVE tensor_tensor_scan (rows 0:64 = num, rows 64:128 =
    den), sigmoid folded into the denominator via e^{-r}, and a partial
    output projection (wkvsr @ Wo[:, shard].T) in bf16. The host sums the 4
    partial outputs per batch.
"""

import ml_dtypes
import numpy as np

import concourse.bass as bass
import concourse.mybir as mybir
import concourse.tile as tile
from concourse import bacc
from concourse.bass_utils import run_bass_kernel_spmd

B, T, D = 2, 512, 256
NCORES = 8
GROUPS = NCORES // B   # 4 channel groups
CLOC = D // GROUPS     # 64 channels per core
PP = 2 * CLOC          # 128 scan rows: num block then den block
F32 = mybir.dt.float32
BF16 = mybir.dt.bfloat16


def build_nc():
    nc = bacc.Bacc(None, target_bir_lowering=False)

    H = T + 1
    xT = nc.declare_dram_parameter("xT", [D, H], BF16, isOutput=False)
    wkvr = nc.declare_dram_parameter("wkvr", [2 * D, 3 * CLOC], BF16, isOutput=False)
    woT = nc.declare_dram_parameter("woT", [CLOC, D], BF16, isOutput=False)
    ab = nc.declare_dram_parameter("abcast", [PP, 1], F32, isOutput=False)
    etf = nc.declare_dram_parameter("etf", [PP, 1], F32, isOutput=False)
    lnd = nc.declare_dram_parameter("lastnd", [PP, 1], F32, isOutput=False)
    outp = nc.declare_dram_parameter("out_part", [T, D], BF16, isOutput=True)
    ndl = nc.declare_dram_parameter("nd_last", [PP, 1], F32, isOutput=True)

    Exp = mybir.ActivationFunctionType.Exp
    mult = mybir.AluOpType.mult
    add = mybir.AluOpType.add

    with tile.TileContext(nc) as tc:
        with (
            tc.tile_pool(name="sb", bufs=1) as sb,
            tc.tile_pool(name="psk", bufs=2, space="PSUM") as psk,
            tc.tile_pool(name="pso", bufs=2, space="PSUM") as pso,
        ):
            # ---- input DMAs across the three DMA queues ----
            # weight lhsT, split so the k/v group (matmul 1) lands first
            wr = wkvr.rearrange("(k p) m -> p k m", k=4)
            wtA = sb.tile([128, 4, 2 * CLOC], BF16)
            for kt in (0, 2, 1, 3):  # match matmul consumption order
                nc.scalar.dma_start(out=wtA[:, kt, :],
                                    in_=wr[:, kt, 0 : 2 * CLOC])
            xt0 = sb.tile([128, H], BF16)
            xt1 = sb.tile([128, H], BF16)
            nc.sync.dma_start(out=xt0[0:64, :], in_=xT[0:64, :])
            nc.gpsimd.dma_start(out=xt0[64:128, :], in_=xT[64:128, :])
            nc.sync.dma_start(out=xt1[0:64, :], in_=xT[128:192, :])
            nc.gpsimd.dma_start(out=xt1[64:128, :], in_=xT[192:256, :])
            xt = [xt0, xt1]
            wtB = sb.tile([128, 4, CLOC], BF16)
            nc.scalar.dma_start(out=wtB, in_=wr[:, :, 2 * CLOC : 3 * CLOC])
            ab_t = sb.tile([PP, 1], F32)
            nc.sync.dma_start(out=ab_t, in_=ab[:, :])
            wo_t = sb.tile([CLOC, D], BF16)
            nc.scalar.dma_start(out=wo_t, in_=woT[:, :])
            etf_t = sb.tile([PP, 1], F32)
            nc.scalar.dma_start(out=etf_t, in_=etf[:, :])

            # working tiles:
            #   u: rows 0:64 = e^k * v (num input), rows 64:128 = e^k (den)
            #   ndbuf col 0 = incoming state, cols 1..T = scanned num/den
            u = sb.tile([PP, T], F32)
            ndbuf = sb.tile([PP, T + 1], F32)
            er64 = sb.tile([CLOC, T], F32)   # e^{-r}
            nmrT = sb.tile([CLOC, T], F32)
            dnmT = sb.tile([CLOC, T], F32)
            dt = sb.tile([CLOC, T], F32)     # dnm * (1 + e^{-r})
            rcp = sb.tile([CLOC, T], F32)
            wsrb = sb.tile([CLOC, T], BF16)

            nc.gpsimd.dma_start(out=ndbuf[:, 0:1], in_=lnd[:, :])

            # warm the Exp table while DMAs run
            dummy = sb.tile([1, 1], F32)
            nc.vector.memset(dummy, 0.0)
            nc.scalar.activation(dummy, dummy, Exp)

            # ---- k/v and r projection matmuls (bf16 in, f32 accumulate) ----
            psA = psk.tile([2 * CLOC, T], F32, tag="kv")  # rows 0:64 k, 64:128 v
            psB = psk.tile([CLOC, T], F32, tag="r")
            # xt0-consuming matmuls first: xt1 halves land on the queues'
            # second slots, so kt order 0,2 (xt0) then 1,3 (xt1)
            KT_ORDER = (0, 2, 1, 3)
            for i, kt in enumerate(KT_ORDER):
                shift = 1 if kt < 2 else 0  # x view vs time-shifted xs view
                rhs = xt[kt % 2][:, shift : shift + T]
                nc.tensor.matmul(psA, wtA[:, kt, :], rhs,
                                 start=(i == 0), stop=(i == 3))
            for i, kt in enumerate(KT_ORDER):
                shift = 1 if kt < 2 else 0
                rhs = xt[kt % 2][:, shift : shift + T]
                nc.tensor.matmul(psB, wtB[:, kt, :], rhs,
                                 start=(i == 0), stop=(i == 3))

            # ---- activations: all on the Exp table ----
            nc.scalar.activation(u[CLOC:PP, :], psA[0:CLOC, :], Exp)  # e^k
            nc.scalar.activation(er64, psB[:, :], Exp, scale=-1.0)    # e^-r

            # u_num = e^k * v  (single 64-row op, SBUF read at base 64 with
            # a PSUM partner; falls back to 2x32 rows if HW disagrees)
            nc.vector.tensor_mul(u[0:CLOC, :], u[CLOC:PP, :], psA[CLOC:PP, :])

            # ---- one 128-row scan does num AND den ----
            nc.vector.tensor_tensor_scan(
                ndbuf[:, 1 : T + 1], ab_t.to_broadcast((PP, T)), u,
                ndbuf[:, 0:1], mult, add)
            # dnm = ek*e^tf + den_prev (reads base 64, writes base 0)
            nc.vector.scalar_tensor_tensor(
                dnmT, u[CLOC:PP, :], etf_t[CLOC:PP, 0:1],
                ndbuf[CLOC:PP, 0:T], mult, add)
            # dt = dnm * (1 + e^-r): folds the sigmoid into the denominator
            nc.vector.scalar_tensor_tensor(dt, er64, 1.0, dnmT, add, mult)
            nc.vector.reciprocal_approx_fast(rcp, dt)
            nc.vector.scalar_tensor_tensor(
                nmrT, u[0:CLOC, :], etf_t[0:CLOC, 0:1],
                ndbuf[0:CLOC, 0:T], mult, add)
            nc.vector.tensor_mul(wsrb, nmrT, rcp)

            # ---- partial output projection (bf16, two t-blocks per bank) ----
            out_dma = [nc.sync, nc.scalar]
            for tp in range(2):
                po = pso.tile([128, 2, D], F32, tag="po")
                for h in range(2):
                    tt = tp * 2 + h
                    nc.tensor.matmul(
                        po[:, h, :], wsrb[:, tt * 128 : (tt + 1) * 128],
                        wo_t, start=True, stop=True,
                    )
                ob = sb.tile([128, 2, D], BF16, tag=f"ob{tp}")
                nc.any.tensor_copy(ob, po)
                for h in range(2):
                    tt = tp * 2 + h
                    out_dma[h].dma_start(
                        out=outp[tt * 128 : (tt + 1) * 128, :],
                        in_=ob[:, h, :],
                    )

            nc.scalar.dma_start(out=ndl[:, 0:1], in_=ndbuf[:, T : T + 1])

    nc.compile()
    return nc


_NC_CACHE = None


def _get_nc():
    global _NC_CACHE
    if _NC_CACHE is None:
        _NC_CACHE = build_nc()
    return _NC_CACHE


def prepare_in_maps(inputs):
    x = np.asarray(inputs["x"], np.float32)
    last_x = np.asarray(inputs["last_x"], np.float32)
    last_num = np.asarray(inputs["last_num"], np.float32)
    last_den = np.asarray(inputs["last_den"], np.float32)
    td = np.asarray(inputs["time_decay"], np.float32)
    tf = np.asarray(inputs["time_first"], np.float32)
    mk = np.asarray(inputs["time_mix_k"], np.float32).reshape(D)
    mv = np.asarray(inputs["time_mix_v"], np.float32).reshape(D)
    mr = np.asarray(inputs["time_mix_r"], np.float32).reshape(D)
    Wk = np.asarray(inputs["Wk"], np.float32)
    Wv = np.asarray(inputs["Wv"], np.float32)
    Wr = np.asarray(inputs["Wr"], np.float32)
    Wo = np.asarray(inputs["Wo"], np.float32)

    # per-batch x^T with the time-shift boundary column
    xTs = []
    for b in range(B):
        xT = np.empty((D, T + 1), np.float32)
        xT[:, 0] = last_x[b, 0, :]
        xT[:, 1:] = x[b].T
        xTs.append(xT.astype(ml_dtypes.bfloat16))

    a = np.exp(-np.exp(td, dtype=np.float64)).astype(np.float32)
    etf_full = np.exp(tf).astype(np.float32)

    in_maps = []
    for c in range(NCORES):
        b = c // GROUPS
        g = c % GROUPS
        sh = slice(g * CLOC, (g + 1) * CLOC)
        wx = np.concatenate(
            [(Wk[sh, :] * mk[None, :]).T,
             (Wv[sh, :] * mv[None, :]).T,
             (Wr[sh, :] * mr[None, :]).T], axis=1)
        wxs = np.concatenate(
            [(Wk[sh, :] * (1.0 - mk)[None, :]).T,
             (Wv[sh, :] * (1.0 - mv)[None, :]).T,
             (Wr[sh, :] * (1.0 - mr)[None, :]).T], axis=1)
        a_pp = np.tile(a[sh], 2)
        etf_pp = np.tile(etf_full[sh], 2)
        lnd_pp = np.concatenate([last_num[b, 0, sh], last_den[b, 0, sh]])
        in_maps.append({
            "xT": xTs[b],
            "wkvr": np.ascontiguousarray(
                np.concatenate([wx, wxs], axis=0)).astype(ml_dtypes.bfloat16),
            "woT": np.ascontiguousarray(Wo[:, sh].T).astype(ml_dtypes.bfloat16),
            "abcast": np.ascontiguousarray(a_pp[:, None], dtype=np.float32),
            "etf": np.ascontiguousarray(etf_pp[:, None], dtype=np.float32),
            "lastnd": np.ascontiguousarray(lnd_pp[:, None], dtype=np.float32),
        })
    return in_maps


def postprocess(results, inputs):
    x = np.asarray(inputs["x"], np.float32)
    out = np.zeros((B, T, D), np.float32)
    num_l = np.empty((B, 1, D), np.float32)
    den_l = np.empty((B, 1, D), np.float32)
    for c in range(NCORES):
        b = c // GROUPS
        g = c % GROUPS
        sh = slice(g * CLOC, (g + 1) * CLOC)
        out[b] += results[c]["out_part"].reshape(T, D).astype(np.float32)
        nd = np.asarray(results[c]["nd_last"], np.float32).reshape(PP)
        num_l[b, 0, sh] = nd[0:CLOC]
        den_l[b, 0, sh] = nd[CLOC:PP]
    x_last = np.ascontiguousarray(x[:, -1:, :])
    return out, x_last, num_l, den_l


def kernel(**inputs):
    nc = _get_nc()
    in_maps = prepare_in_maps(inputs)
    res = run_bass_kernel_spmd(nc, in_maps, list(range(NCORES)))
    return postprocess(res.results, inputs)
